# revision 1
# baseline (speedup 1.0000x reference)
"""Trainium2 Bass kernel for nn_InterfaceGraph (retrieval_knn).

Segment-restricted nearest neighbors between pos_a and pos_b (16384 x
16384 pairwise distances, block-diagonal over 64 sorted graphs), sharded
over 8 NeuronCores (8 graphs per core, slot-sorted by size so the SPMD
program's per-slot shapes stay tight).

Per 128-row tile of a graph block, one bf16 matmul (K=21: a bf16x3
split of 2*a.b - |b|^2, small terms accumulated first) writes the
negated-distance key into PSUM at full speed; VectorE max/max_index read
PSUM directly and produce the row min + first-occurrence argmin, exactly
matching fp32 argmin semantics to ~1-2 ulp (validated: zero flips vs the
fp32 reference on the target data).  |a|^2 is omitted: it is constant
along the scanned axis, so it cannot change the argmin.  Both directions
(a->b, b->a) are computed the same way.

Host does the O(N) epilogue: gather + norm (same arithmetic as the
reference), residue segment-max interface mask, mutation OR, concat.
"""

import numpy as np
import ml_dtypes

NCORES = 8
G = 64
GPC = G // NCORES
NUM_RESIDUES = 2048
CUTOFF = np.float32(10.0)
BIG = np.float32(2.0 ** 26)
K = 21            # 9 tier-2 + 6 tier-1 + 3 tier-0 cross rows + 3 |b|^2 rows

PROFILE = False
LAST_EXEC_NS = None

BF16 = ml_dtypes.bfloat16

_prog_cache = {}


def _round_up(x, m):
    return (x + m - 1) // m * m


def _install_ntff_hook():
    import sys
    import types
    if 'antenv.axon_hooks' in sys.modules:
        return
    from trn_agent_boot.trn_boot import _ntff_profile_via_ctypes
    hook = _ntff_profile_via_ctypes('/opt/axon/libaxon_pjrt.so')
    mod = types.ModuleType('antenv.axon_hooks')
    mod.get_axon_ntff_profile_hook = lambda: hook
    sys.modules['antenv.axon_hooks'] = mod


def _split3(v):
    """bf16x3 split: v ~= v1 + v2 + v3 with ~24-bit mantissa coverage."""
    v = v.astype(np.float32)
    v1 = v.astype(BF16).astype(np.float32)
    r = v - v1
    v2 = r.astype(BF16).astype(np.float32)
    v3 = (r - v2).astype(BF16).astype(np.float32)
    return v1, v2, v3


class _Geom:
    """Per-slot shapes shared by all cores (SPMD program is one program).

    Slot assignment is independent per side: A-side slots sort each
    core's graphs by na desc (tile count), B-side by nb desc, which
    keeps the cross-core per-slot maxima tight.
    """

    def __init__(self, na, nb):
        gid = (np.arange(NCORES * GPC).reshape(NCORES, GPC) // GPC) * GPC
        ordA = np.zeros((NCORES, GPC), dtype=np.int64)
        ordB = np.zeros((NCORES, GPC), dtype=np.int64)
        for c in range(NCORES):
            loc = np.arange(GPC)
            ordA[c] = loc[np.argsort(-na[c * GPC + loc], kind="stable")]
            ordB[c] = loc[np.argsort(-nb[c * GPC + loc], kind="stable")]
        self.graphA = gid + ordA               # [core, slot] -> graph id
        self.graphB = gid + ordB
        na_A = na[self.graphA]
        nb_A = nb[self.graphA]
        nb_B = nb[self.graphB]
        na_B = na[self.graphB]
        self.TA = [int(-(-na_A[:, s].max() // 128)) for s in range(GPC)]
        self.TB = [int(-(-nb_B[:, s].max() // 128)) for s in range(GPC)]
        self.WB = [int(max(8, _round_up(int(nb_A[:, s].max()), 4)))
                   for s in range(GPC)]
        self.WA = [int(max(8, _round_up(int(na_B[:, s].max()), 4)))
                   for s in range(GPC)]
        self.baseTA = np.concatenate([[0], np.cumsum(self.TA)]).astype(int)
        self.baseTB = np.concatenate([[0], np.cumsum(self.TB)]).astype(int)
        self.baseWB = np.concatenate([[0], np.cumsum(self.WB)]).astype(int)
        self.baseWA = np.concatenate([[0], np.cumsum(self.WA)]).astype(int)

    def key(self):
        return (tuple(self.TA), tuple(self.TB), tuple(self.WB), tuple(self.WA))


def _build_program(geom):
    from contextlib import ExitStack

    import concourse.bacc as bacc
    import concourse.mybir as mybir
    import concourse.tile as tile

    f32 = mybir.dt.float32
    bf16 = mybir.dt.bfloat16
    u32 = mybir.dt.uint32

    LA = int(geom.baseTA[-1]) * 128   # lhsA columns
    LB = int(geom.baseTB[-1]) * 128
    RB = int(geom.baseWB[-1])         # rhsB columns
    RA = int(geom.baseWA[-1])
    OA = int(geom.baseTA[-1]) * 8     # output columns, a-side
    OB = int(geom.baseTB[-1]) * 8

    nc = bacc.Bacc("TRN2", target_bir_lowering=False, debug=False,
                   enable_asserts=True, num_devices=NCORES)

    lhsA = nc.dram_tensor("lhsA", [K, LA], bf16, kind="ExternalInput").ap()
    rhsB = nc.dram_tensor("rhsB", [K, RB], bf16, kind="ExternalInput").ap()
    lhsB = nc.dram_tensor("lhsB", [K, LB], bf16, kind="ExternalInput").ap()
    rhsA = nc.dram_tensor("rhsA", [K, RA], bf16, kind="ExternalInput").ap()
    idxA = nc.dram_tensor("idxA", [128, OA], u32, kind="ExternalOutput").ap()
    idxB = nc.dram_tensor("idxB", [128, OB], u32, kind="ExternalOutput").ap()

    with tile.TileContext(nc) as tc:
        with ExitStack() as ctx:
            const = ctx.enter_context(tc.tile_pool(name="const", bufs=1))
            psum = ctx.enter_context(
                tc.tile_pool(name="psum", bufs=8, space="PSUM"))
            work = ctx.enter_context(tc.tile_pool(name="work", bufs=6))

            lhsA_sb = const.tile([K, LA], bf16, tag="lhsA")
            nc.sync.dma_start(lhsA_sb[:], lhsA[:])
            rhsB_sb = const.tile([K, RB], bf16, tag="rhsB")
            nc.sync.dma_start(rhsB_sb[:], rhsB[:])
            lhsB_sb = const.tile([K, LB], bf16, tag="lhsB")
            nc.sync.dma_start(lhsB_sb[:], lhsB[:])
            rhsA_sb = const.tile([K, RA], bf16, tag="rhsA")
            nc.sync.dma_start(rhsA_sb[:], rhsA[:])

            valA_sb = const.tile([128, OA], f32, tag="valA")
            idxA_sb = const.tile([128, OA], u32, tag="idxA")
            valB_sb = const.tile([128, OB], f32, tag="valB")
            idxB_sb = const.tile([128, OB], u32, tag="idxB")

            def side(lhs_sb, rhs_sb, T, baseT, W, baseW, val_sb, idx_sb):
                for s in range(GPC):
                    for t in range(T[s]):
                        kk = int(baseT[s]) + t
                        ps = psum.tile([128, W[s]], f32, tag="ps")
                        nc.tensor.matmul(
                            ps[:],
                            lhs_sb[:, kk * 128:(kk + 1) * 128],
                            rhs_sb[:, int(baseW[s]):int(baseW[s]) + W[s]],
                            start=True, stop=True)
                        # VectorE max/max_index read PSUM directly (measured
                        # same per-op cost as SBUF; skipping the ScalarE
                        # copy shortens each tile's dependency chain).
                        nc.vector.max(val_sb[:, kk * 8:(kk + 1) * 8], ps[:])
                        nc.vector.max_index(
                            idx_sb[:, kk * 8:(kk + 1) * 8],
                            val_sb[:, kk * 8:(kk + 1) * 8], ps[:])

            side(lhsA_sb, rhsB_sb, geom.TA, geom.baseTA,
                 geom.WB, geom.baseWB, valA_sb, idxA_sb)
            side(lhsB_sb, rhsA_sb, geom.TB, geom.baseTB,
                 geom.WA, geom.baseWA, valB_sb, idxB_sb)

            nc.sync.dma_start(idxA[:], idxA_sb[:])
            nc.sync.dma_start(idxB[:], idxB_sb[:])

    nc.compile()
    return nc


def _pack_side(pos_row, pos_col, starts_row, starts_col, graphs,
               T, baseT, W, baseW):
    """lhs/rhs bf16 packs for one core, one direction.

    Row side (stationary): coords doubled, bf16x3 split.
    Col side (moving): coords + |q|^2 split; key = 2 p.q - |q|^2.
    K-row order: tier-2 (smallest) first, tier-0 last.
    """
    LT = int(baseT[-1]) * 128
    RW = int(baseW[-1])
    lhs = np.zeros((K, LT), dtype=np.float32)
    rhs = np.zeros((K, RW), dtype=np.float32)
    # q-split rows: tier2 row 9, tier1 rows 15-16?  layout below:
    #  rows 0-8   : tier2 cross (c,x3) lhs a1,a2,a3 / rhs b3,b2,b1
    #  row  9     : tier2 -q3      (lhs -1, rhs q3)
    #  rows 10-15 : tier1 cross    lhs a1,a2 / rhs b2,b1
    #  row  16    : tier1 -q2
    #  rows 17-19 : tier0 cross    lhs a1 / rhs b1
    #  row  20    : tier0 -q1  (+BIG on padding)
    lhs[9, :] = -1.0
    lhs[16, :] = -1.0
    lhs[20, :] = -1.0
    rhs[20, :] = BIG  # padding columns lose every argmax
    for s in range(GPC):
        g = graphs[s]
        p = pos_row[starts_row[g]:starts_row[g + 1]]
        n = p.shape[0]
        lb = int(baseT[s]) * 128
        for c in range(3):
            a1, a2, a3 = _split3(np.float32(2.0) * p[:, c])
            lhs[0 + c * 3, lb:lb + n] = a1
            lhs[1 + c * 3, lb:lb + n] = a2
            lhs[2 + c * 3, lb:lb + n] = a3
            lhs[10 + c * 2, lb:lb + n] = a1
            lhs[11 + c * 2, lb:lb + n] = a2
            lhs[17 + c, lb:lb + n] = a1
        # padding rows: zero coords, and kill the -1 rows so pad rows
        # read 0 - (-BIG)?  (pad rows' outputs are discarded anyway)

        q = pos_col[starts_col[g]:starts_col[g + 1]]
        m = q.shape[0]
        rb = int(baseW[s])
        qq = (q[:, 0] * q[:, 0] + q[:, 1] * q[:, 1]) + q[:, 2] * q[:, 2]
        q1, q2, q3 = _split3(qq)
        for c in range(3):
            b1, b2, b3 = _split3(q[:, c])
            rhs[0 + c * 3, rb:rb + m] = b3
            rhs[1 + c * 3, rb:rb + m] = b2
            rhs[2 + c * 3, rb:rb + m] = b1
            rhs[10 + c * 2, rb:rb + m] = b2
            rhs[11 + c * 2, rb:rb + m] = b1
            rhs[17 + c, rb:rb + m] = b1
        rhs[9, rb:rb + m] = q3
        rhs[16, rb:rb + m] = q2
        rhs[20, rb:rb + m] = q1
    return lhs.astype(BF16), rhs.astype(BF16)


def _unpack_side(res_idx, starts_row, starts_col, graphs, baseT, idx_full):
    for s in range(GPC):
        g = graphs[s]
        n = starts_row[g + 1] - starts_row[g]
        for t in range((n + 127) // 128):
            rows = min(128, n - t * 128)
            kk = int(baseT[s]) + t
            loc = res_idx[:rows, kk * 8].astype(np.int64)
            atoms = starts_row[g] + t * 128 + np.arange(rows)
            idx_full[atoms] = starts_col[g] + loc


def kernel(pos_a, pos_b, node2graph_a, node2graph_b,
           atom2residue_a, atom2residue_b, is_mutation):
    global LAST_EXEC_NS

    from concourse.bass_utils import run_bass_kernel_spmd

    pos_a = np.asarray(pos_a, dtype=np.float32)
    pos_b = np.asarray(pos_b, dtype=np.float32)
    node2graph_a = np.asarray(node2graph_a)
    node2graph_b = np.asarray(node2graph_b)
    atom2residue_a = np.asarray(atom2residue_a)
    atom2residue_b = np.asarray(atom2residue_b)
    is_mutation = np.asarray(is_mutation)

    Na = pos_a.shape[0]
    Nb = pos_b.shape[0]

    sa = np.searchsorted(node2graph_a, np.arange(G + 1)).astype(np.int64)
    sb = np.searchsorted(node2graph_b, np.arange(G + 1)).astype(np.int64)
    na = np.diff(sa)
    nb = np.diff(sb)
    assert na.min() > 0 and nb.min() > 0, "empty graph block not supported"

    geom = _Geom(na, nb)
    key = geom.key()
    if key not in _prog_cache:
        _prog_cache[key] = _build_program(geom)
    nc = _prog_cache[key]

    in_maps = []
    for c in range(NCORES):
        lhsA, rhsB = _pack_side(pos_a, pos_b, sa, sb, geom.graphA[c],
                                geom.TA, geom.baseTA, geom.WB, geom.baseWB)
        lhsB, rhsA = _pack_side(pos_b, pos_a, sb, sa, geom.graphB[c],
                                geom.TB, geom.baseTB, geom.WA, geom.baseWA)
        in_maps.append({"lhsA": lhsA, "rhsB": rhsB,
                        "lhsB": lhsB, "rhsA": rhsA})

    if PROFILE:
        _install_ntff_hook()
    res = run_bass_kernel_spmd(nc, in_maps, list(range(NCORES)),
                               trace=bool(PROFILE))
    if PROFILE:
        LAST_EXEC_NS = res.exec_time_ns

    idx_a = np.zeros(Na, dtype=np.int64)
    idx_b = np.zeros(Nb, dtype=np.int64)
    for c in range(NCORES):
        _unpack_side(res.results[c]["idxA"], sa, sb, geom.graphA[c],
                     geom.baseTA, idx_a)
        _unpack_side(res.results[c]["idxB"], sb, sa, geom.graphB[c],
                     geom.baseTB, idx_b)

    da = pos_a - pos_b[idx_a]
    dist_a = np.sqrt((da[:, 0] * da[:, 0] + da[:, 1] * da[:, 1])
                     + da[:, 2] * da[:, 2])
    db = pos_b - pos_a[idx_b]
    dist_b = np.sqrt((db[:, 0] * db[:, 0] + db[:, 1] * db[:, 1])
                     + db[:, 2] * db[:, 2])

    def iface_mask(dist, atom2residue):
        is_if = (dist < CUTOFF).astype(np.int32)
        res_max = np.zeros(NUM_RESIDUES, dtype=np.int32)
        np.maximum.at(res_max, atom2residue, is_if)
        return res_max[atom2residue] > 0

    mask_a = iface_mask(dist_a, atom2residue_a)
    mask_b = iface_mask(dist_b, atom2residue_b)
    mask = np.concatenate([mask_a, mask_b]) | is_mutation.astype(bool)
    dists = np.concatenate([dist_a, dist_b]).astype(np.float32)
    return mask, dists



# revision 5
# speedup vs baseline: 1.6115x; 1.6115x over previous
"""Trainium2 Bass kernel for nn_InterfaceGraph (retrieval_knn).

Value-only formulation: the reference's outputs (mask, dists) depend only
on each atom's MINIMUM same-graph distance, not on which neighbor attains
it.  So the device computes, per atom, max_j key_ij where key = -d^2 is
produced directly by one bf16 matmul per 128-row tile (K=36 rows: per
coordinate, split3 cross terms plus |a_c|^2 / |b_c|^2 rows, ordered so
fp32 PSUM accumulation cancels early; max abs d^2 error 0.014 on the
target data).  VectorE does ONE slab tensor_reduce(max) per 4-bank PSUM
group -- no FIND_INDEX8 / MAX8 passes at all, which removes ~60% of the
baseline's DVE time.

Host epilogue: d = sqrt(-max); rows with d < 2.5 or |d-10| < 0.6 (~11%)
are recomputed exactly (fp64 argmin + fp32 norm, matching the reference
formula) so small-d relative error and the d<10 interface-cutoff
comparisons are exact; residue segment-max mask + mutation OR as before.

Sharding: all 316 row-tiles (both directions) are sorted by column width
and dealt round-robin to the 8 cores, so per-slot cross-core widths are
tight (SPMD program shapes are cross-core maxima).
"""

import numpy as np
import ml_dtypes

NCORES = 8
G = 64
NUM_RESIDUES = 2048
CUTOFF = np.float32(10.0)
BIG = np.float32(2.0 ** 26)
K = 36
GROUP = 4          # psum banks (tiles) per reduce slab

PROFILE = False
LAST_EXEC_NS = None

BF16 = ml_dtypes.bfloat16

_prog_cache = {}


def _round_up(x, m):
    return (x + m - 1) // m * m


def _install_ntff_hook():
    import sys
    import types
    if 'antenv.axon_hooks' in sys.modules:
        return
    from trn_agent_boot.trn_boot import _ntff_profile_via_ctypes
    hook = _ntff_profile_via_ctypes('/opt/axon/libaxon_pjrt.so')
    mod = types.ModuleType('antenv.axon_hooks')
    mod.get_axon_ntff_profile_hook = lambda: hook
    sys.modules['antenv.axon_hooks'] = mod


def _split3(v):
    v = v.astype(np.float32)
    v1 = v.astype(BF16).astype(np.float32)
    r = v - v1
    v2 = r.astype(BF16).astype(np.float32)
    v3 = (r - v2).astype(BF16).astype(np.float32)
    return v1, v2, v3


class _Geom:
    """Tile lists and per-slot shapes for one side (row->col direction).

    A tile is 128 consecutive row-atoms of one graph scanning that
    graph's full column block.  Tiles from BOTH sides... (this class is
    one side; kernel builds two).  Sorted by padded column width desc,
    slot s holds tiles [8s:8s+8] across the 8 cores; missing entries are
    dummy tiles (zero lhs).
    """

    def __init__(self, n_row, n_col):
        tiles = []          # (graph, row_chunk, W)
        for g in range(G):
            W = max(8, _round_up(int(n_col[g]), 4))
            for r in range(-(-int(n_row[g]) // 128)):
                tiles.append((g, r, W))
        tiles.sort(key=lambda t: (-t[2], t[0], t[1]))
        self.nslots = -(-len(tiles) // NCORES)
        # pad to full slots with dummies (graph=-1)
        tiles += [(-1, 0, 8)] * (self.nslots * NCORES - len(tiles))
        self.ngroups = -(-self.nslots // GROUP)
        tiles += [(-1, 0, 8)] * ((self.ngroups * GROUP - self.nslots) * NCORES)
        self.nslots = self.ngroups * GROUP
        # slot s, core c -> tiles[s*8 + c]
        self.tile = [[tiles[s * NCORES + c] for c in range(NCORES)]
                     for s in range(self.nslots)]
        self.Wgrp = []
        for grp in range(self.ngroups):
            w = max(self.tile[s][c][2]
                    for s in range(grp * GROUP,
                                   min((grp + 1) * GROUP, self.nslots))
                    for c in range(NCORES))
            self.Wgrp.append(int(_round_up(w, 4)))
        self.L = self.nslots * 128                 # lhs columns
        # rhs windows: dedup per (graph, Wgrp) per core
        self.rhs_cols = [GROUP * w for w in self.Wgrp]
        self.rhs_base = np.concatenate(
            [[0], np.cumsum(self.rhs_cols)]).astype(int)
        self.R = int(self.rhs_base[-1])

    def key(self):
        return (self.nslots, tuple(self.Wgrp))


def _build_program(gA, gB):
    from contextlib import ExitStack

    import concourse.bacc as bacc
    import concourse.mybir as mybir
    import concourse.tile as tile

    f32 = mybir.dt.float32
    bf16 = mybir.dt.bfloat16

    nc = bacc.Bacc("TRN2", target_bir_lowering=False, debug=False,
                   enable_asserts=True, num_devices=NCORES)

    lhsA = nc.dram_tensor("lhsA", [K, gA.L], bf16, kind="ExternalInput").ap()
    rhsA = nc.dram_tensor("rhsA", [K, gA.R], bf16, kind="ExternalInput").ap()
    lhsB = nc.dram_tensor("lhsB", [K, gB.L], bf16, kind="ExternalInput").ap()
    rhsB = nc.dram_tensor("rhsB", [K, gB.R], bf16, kind="ExternalInput").ap()
    valA = nc.dram_tensor("valA", [128, gA.nslots], f32,
                          kind="ExternalOutput").ap()
    valB = nc.dram_tensor("valB", [128, gB.nslots], f32,
                          kind="ExternalOutput").ap()

    with tile.TileContext(nc) as tc:
        with ExitStack() as ctx:
            const = ctx.enter_context(tc.tile_pool(name="const", bufs=1))
            psum = ctx.enter_context(
                tc.tile_pool(name="psum", bufs=2, space="PSUM"))

            lhsA_sb = const.tile([K, gA.L], bf16, tag="lhsA")
            rhsA_sb = const.tile([K, gA.R], bf16, tag="rhsA")
            lhsB_sb = const.tile([K, gB.L], bf16, tag="lhsB")
            rhsB_sb = const.tile([K, gB.R], bf16, tag="rhsB")
            valA_sb = const.tile([128, gA.nslots], f32, tag="valA")
            valB_sb = const.tile([128, gB.nslots], f32, tag="valB")

            # stage DMAs: first group's worth first so PE starts early
            cut_lA = min(GROUP, gA.nslots) * 128
            cut_rA = int(gA.rhs_base[1])
            nc.sync.dma_start(lhsA_sb[:, :cut_lA], lhsA[:, :cut_lA])
            nc.sync.dma_start(rhsA_sb[:, :cut_rA], rhsA[:, :cut_rA])
            nc.sync.dma_start(lhsA_sb[:, cut_lA:], lhsA[:, cut_lA:])
            nc.sync.dma_start(rhsA_sb[:, cut_rA:], rhsA[:, cut_rA:])
            nc.sync.dma_start(lhsB_sb[:], lhsB[:])
            nc.sync.dma_start(rhsB_sb[:], rhsB[:])

            def side(geom, lhs_sb, rhs_sb, val_sb):
                for grp in range(geom.ngroups):
                    W = geom.Wgrp[grp]
                    ps = psum.tile([128, GROUP, 512], f32, tag="ps")
                    for k in range(GROUP):
                        s = grp * GROUP + k
                        off = int(geom.rhs_base[grp]) + k * W
                        nc.tensor.matmul(
                            ps[:, k, 0:W],
                            lhs_sb[:, s * 128:(s + 1) * 128],
                            rhs_sb[:, off:off + W],
                            start=True, stop=True)
                    nc.vector.reduce_max(
                        val_sb[:, grp * GROUP:(grp + 1) * GROUP],
                        ps[:, :, 0:W], axis=mybir.AxisListType.X)

            side(gA, lhsA_sb, rhsA_sb, valA_sb)
            nc.sync.dma_start(valA[:], valA_sb[:])
            side(gB, lhsB_sb, rhsB_sb, valB_sb)
            nc.sync.dma_start(valB[:], valB_sb[:])

    nc.compile()
    return nc


def _pack_side(geom, pos_row, pos_col, starts_row, starts_col, core):
    """lhs [K, L] / rhs [K, R] bf16 for one core, one side."""
    lhs = np.zeros((K, geom.L), np.float32)
    rhs = np.zeros((K, geom.R), np.float32)
    rhs[2, :] = BIG                    # default: every rhs col loses the max
    for grp in range(geom.ngroups):
        W = geom.Wgrp[grp]
        for k in range(GROUP):
            s = grp * GROUP + k
            g, r, _ = geom.tile[s][core]
            if g < 0:
                continue
            lb = s * 128
            p = pos_row[starts_row[g] + 128 * r:
                        min(starts_row[g] + 128 * (r + 1), starts_row[g + 1])]
            n = p.shape[0]
            off = int(geom.rhs_base[grp]) + k * W
            q = pos_col[starts_col[g]:starts_col[g + 1]]
            m = q.shape[0]
            for c in range(3):
                base = c * 12
                u1, u2, u3 = _split3(q[:, c])
                v1, v2, v3 = _split3(q[:, c] * q[:, c])
                sl = slice(off, off + m)
                rhs[base + 0, sl] = 1.0
                rhs[base + 1, sl] = u1
                rhs[base + 2, sl] = v1
                rhs[base + 3, sl] = 1.0
                rhs[base + 4, sl] = u2
                rhs[base + 5, sl] = u1
                rhs[base + 6, sl] = v2
                rhs[base + 7, sl] = 1.0
                rhs[base + 8, sl] = u3
                rhs[base + 9, sl] = u2
                rhs[base + 10, sl] = u1
                rhs[base + 11, sl] = v3
            rhs[2, off + m:off + W] = BIG
            for c in range(3):
                base = c * 12
                t = p[:, c]
                s1, s2, s3 = _split3(t * t)
                t21, t22, t23 = _split3(np.float32(2.0) * t)
                sl = slice(lb, lb + n)
                lhs[base + 0, sl] = -s1
                lhs[base + 1, sl] = t21
                lhs[base + 2, sl] = -1.0
                lhs[base + 3, sl] = -s2
                lhs[base + 4, sl] = t21
                lhs[base + 5, sl] = t22
                lhs[base + 6, sl] = -1.0
                lhs[base + 7, sl] = -s3
                lhs[base + 8, sl] = t21
                lhs[base + 9, sl] = t22
                lhs[base + 10, sl] = t23
                lhs[base + 11, sl] = -1.0
    return lhs.astype(BF16), rhs.astype(BF16)


def kernel(pos_a, pos_b, node2graph_a, node2graph_b,
           atom2residue_a, atom2residue_b, is_mutation):
    global LAST_EXEC_NS

    from concourse.bass_utils import run_bass_kernel_spmd

    pos_a = np.asarray(pos_a, dtype=np.float32)
    pos_b = np.asarray(pos_b, dtype=np.float32)
    node2graph_a = np.asarray(node2graph_a)
    node2graph_b = np.asarray(node2graph_b)
    atom2residue_a = np.asarray(atom2residue_a)
    atom2residue_b = np.asarray(atom2residue_b)
    is_mutation = np.asarray(is_mutation)

    sa = np.searchsorted(node2graph_a, np.arange(G + 1)).astype(np.int64)
    sb = np.searchsorted(node2graph_b, np.arange(G + 1)).astype(np.int64)
    na = np.diff(sa)
    nb = np.diff(sb)
    assert na.min() > 0 and nb.min() > 0, "empty graph block not supported"

    gA = _Geom(na, nb)        # a rows vs b cols
    gB = _Geom(nb, na)        # b rows vs a cols
    key = (gA.key(), gB.key())
    if key not in _prog_cache:
        _prog_cache[key] = _build_program(gA, gB)
    nc = _prog_cache[key]

    in_maps = []
    for c in range(NCORES):
        lhsA, rhsA = _pack_side(gA, pos_a, pos_b, sa, sb, c)
        lhsB, rhsB = _pack_side(gB, pos_b, pos_a, sb, sa, c)
        in_maps.append({"lhsA": lhsA, "rhsA": rhsA,
                        "lhsB": lhsB, "rhsB": rhsB})

    if PROFILE:
        _install_ntff_hook()
    res = run_bass_kernel_spmd(nc, in_maps, list(range(NCORES)),
                               trace=bool(PROFILE))
    if PROFILE:
        LAST_EXEC_NS = res.exec_time_ns

    d2_a = np.empty(pos_a.shape[0], np.float32)
    d2_b = np.empty(pos_b.shape[0], np.float32)
    for c in range(NCORES):
        vA = res.results[c]["valA"]
        vB = res.results[c]["valB"]
        for s in range(gA.nslots):
            g, r, _ = gA.tile[s][c]
            if g < 0:
                continue
            lo = sa[g] + 128 * r
            hi = min(sa[g] + 128 * (r + 1), sa[g + 1])
            d2_a[lo:hi] = -vA[0:hi - lo, s]
        for s in range(gB.nslots):
            g, r, _ = gB.tile[s][c]
            if g < 0:
                continue
            lo = sb[g] + 128 * r
            hi = min(sb[g] + 128 * (r + 1), sb[g + 1])
            d2_b[lo:hi] = -vB[0:hi - lo, s]

    def epilogue(d2dev, pos_row, pos_col, s_col, n2row):
        dist = np.sqrt(np.maximum(d2dev, np.float32(0.0))).astype(np.float32)
        flags = np.where((dist < 2.5) | (np.abs(dist - 10.0) < 0.6))[0]
        for i in flags:
            g = n2row[i]
            Q = pos_col[s_col[g]:s_col[g + 1]]
            df = pos_row[i].astype(np.float64) - Q.astype(np.float64)
            j = int(np.argmin((df * df).sum(-1)))
            diff = (pos_row[i] - Q[j]).astype(np.float32)
            dist[i] = np.float32(np.sqrt(np.float32((diff * diff).sum())))
        return dist

    dist_a = epilogue(d2_a, pos_a, pos_b, sb, node2graph_a)
    dist_b = epilogue(d2_b, pos_b, pos_a, sa, node2graph_b)

    def iface_mask(dist, atom2residue):
        is_if = (dist < CUTOFF).astype(np.int32)
        res_max = np.zeros(NUM_RESIDUES, dtype=np.int32)
        np.maximum.at(res_max, atom2residue, is_if)
        return res_max[atom2residue] > 0

    mask_a = iface_mask(dist_a, atom2residue_a)
    mask_b = iface_mask(dist_b, atom2residue_b)
    mask = np.concatenate([mask_a, mask_b]) | is_mutation.astype(bool)
    dists = np.concatenate([dist_a, dist_b]).astype(np.float32)
    return mask, dists
